# revision 6
# baseline (speedup 1.0000x reference)
"""Multi-head attention (B=4, S=2048, D=1024, H=16) on 8 trn2 NeuronCores.

Sharding: (batch, head-group) -> 8 shards of (1 batch x 8 heads). Zero
cross-core communication: each core computes Q/K/V projections for its 8
heads, full attention over S=2048, and a partial output projection
(row-split Wo); the host sums the two head-group partials per batch.

v2 over the 454us baseline: attention processes HEAD PAIRS with the even
head's K/Q slice on SBUF partitions 0:64 and the odd head's on 64:128.
The two scores matmuls of a pair then carry tile_position (0,0) / (64,0)
(auto-derived from base_partition) and execute CONCURRENTLY on disjoint
PE row groups (measured 1.75x on a microbench), recovering the half-PE
waste of the K=dh=64 contraction. Each pair writes one [128, 1024] PSUM
tile = [scores_hA(512q) | scores_hB(512q)], so the exp ACT count stays
256 (ACT is the pacing engine at ~285us busy). Projections are emitted
ht-major (K) / pair-sliced (V) so each pair's sweep dependencies complete
just ahead of its sweep, with leftover projections + the output
projection filling PE slack under the ACT-paced attention phase.
"""

import numpy as np

import concourse.bass as bass
import concourse.tile as tile
from concourse import bacc, mybir
from concourse.bass_utils import run_bass_kernel_spmd

F32 = mybir.dt.float32
F16 = mybir.dt.float16
AF = mybir.ActivationFunctionType

B, S, D = 4, 2048, 1024
HPC = 8          # heads per core
DHT = 512        # head dims per core (8 * 64)
NDT = D // 128   # 8 d-tiles (contraction tiles for projections)
NHT = DHT // 128  # 4 dh-tiles (= head pairs)
NST = S // 128   # 16 s-tiles
NQB = S // 512   # 4 q-blocks
N_CORES = 8


def build_nc():
    nc = bacc.Bacc(None, target_bir_lowering=False)

    xq = nc.declare_dram_parameter("xq_t", [D, S], F16, isOutput=False)
    xk = nc.declare_dram_parameter("xk_t", [D, S], F16, isOutput=False)
    xv = nc.declare_dram_parameter("xv_t", [D, S], F16, isOutput=False)
    wq = nc.declare_dram_parameter("wq", [D, DHT], F16, isOutput=False)
    wk = nc.declare_dram_parameter("wk", [D, DHT], F16, isOutput=False)
    wv = nc.declare_dram_parameter("wv", [D, DHT], F16, isOutput=False)
    wo = nc.declare_dram_parameter("wo", [DHT, D], F16, isOutput=False)
    bq = nc.declare_dram_parameter("bq", [DHT], F32, isOutput=False)
    bk = nc.declare_dram_parameter("bk", [DHT], F32, isOutput=False)
    bv = nc.declare_dram_parameter("bv", [DHT], F32, isOutput=False)
    ot = nc.declare_dram_parameter("o_t", [D, S], F32, isOutput=True)

    xq_v = xq.rearrange("(t p) s -> p t s", p=128)
    xk_v = xk.rearrange("(t p) s -> p t s", p=128)
    xv_v = xv.rearrange("(t p) s -> p t s", p=128)
    wq_v = wq.rearrange("(t p) n -> p t n", p=128)
    wk_v = wk.rearrange("(t p) n -> p t n", p=128)
    wv_v = wv.rearrange("(t p) n -> p t n", p=128)
    wo_v = wo.rearrange("(t p) n -> p t n", p=128)
    ot_v = ot.rearrange("(t p) s -> t p s", p=128)

    with tile.TileContext(nc) as tc:
        with (
            tc.tile_pool(name="persist", bufs=1) as persist,
            tc.tile_pool(name="pexp_p", bufs=4) as pexp_p,
            tc.tile_pool(name="outp", bufs=3) as outp,
            tc.tile_pool(name="small", bufs=2) as small,
            tc.tile_pool(name="ps_sc", bufs=2, space="PSUM") as ps_sc,
            tc.tile_pool(name="ps_ctx", bufs=2, space="PSUM") as ps_ctx,
            tc.tile_pool(name="ps_o", bufs=2, space="PSUM") as ps_o,
        ):
            KT = persist.tile([128, NHT, S], F16)        # K^T  [dh, s]
            QT = persist.tile([128, NHT, S], F16)        # Q^T  [dh, s]
            Vt = persist.tile([128, NST, HPC, 65], F16)  # V natural + ones
            ctxn = persist.tile([128, NHT, S], F16)      # normalized ctx^T
            wq_sb = persist.tile([128, NDT, DHT], F16)
            wk_sb = persist.tile([128, NDT, DHT], F16)
            wv_sb = persist.tile([128, NDT, DHT], F16)
            wo_sb = persist.tile([128, NHT, D], F16)
            xk_sb = persist.tile([128, NDT, S], F16)
            xv_sb = persist.tile([128, NDT, S], F16)
            xq_sb = persist.tile([128, NDT, 1024], F16)
            bq_sb = persist.tile([128, NHT], F32)
            bk_sb = persist.tile([128, NHT], F32)
            bv_bc = persist.tile([128, HPC, 64], F32)

            # DMA issue spread over 4 engine queues so the prefix
            # transfers parallelize: scalar=wk/wq/wo, vector=wv+biases,
            # sync=xk+xq (block-major), gpsimd=xv (block-major)
            nc.scalar.dma_start(out=bq_sb, in_=bq.rearrange("(t p) -> p t", p=128))
            nc.scalar.dma_start(out=bk_sb, in_=bk.rearrange("(t p) -> p t", p=128))
            nc.scalar.dma_start(
                out=bv_bc,
                in_=bv.rearrange("(h d) -> h d", d=64).partition_broadcast(128),
            )
            nc.vector.memset(Vt[:, :, :, 64:65], 1.0)
            for dt in range(NDT):
                nc.scalar.dma_start(out=wk_sb[:, dt, :], in_=wk_v[:, dt, :])
            for dt in range(NDT):
                nc.gpsimd.dma_start(out=wv_sb[:, dt, :], in_=wv_v[:, dt, :])
            for dt in range(NDT):
                nc.scalar.dma_start(out=wq_sb[:, dt, :], in_=wq_v[:, dt, :])
            for kt in range(NHT):
                nc.scalar.dma_start(out=wo_sb[:, kt, :], in_=wo_v[:, kt, :])

            def dma_xk(b):
                ssl = slice(b * 512, (b + 1) * 512)
                for dt in range(NDT):
                    nc.sync.dma_start(out=xk_sb[:, dt, ssl], in_=xk_v[:, dt, ssl])

            def dma_xv(b):
                ssl = slice(b * 512, (b + 1) * 512)
                for dt in range(NDT):
                    nc.gpsimd.dma_start(out=xv_sb[:, dt, ssl], in_=xv_v[:, dt, ssl])

            def dma_xq(qb):
                ssl = slice(qb * 512, (qb + 1) * 512)
                sb = slice((qb % 2) * 512, (qb % 2) * 512 + 512)
                for dt in range(NDT):
                    nc.sync.dma_start(out=xq_sb[:, dt, sb], in_=xq_v[:, dt, ssl])

            dma_xk(0)
            dma_xq(0)
            dma_xq(1)
            dma_xk(1)
            dma_xk(2)
            dma_xk(3)
            for b in range(4):
                dma_xv(b)

            def emit_k_proj(ht):
                # KT[:, ht, :] for head pair ht (dh rows 0:64 / 64:128)
                for b in range(4):
                    ssl = slice(b * 512, (b + 1) * 512)
                    ps = ps_o.tile([128, 512], F32, tag="po")
                    for dt in range(NDT):
                        nc.tensor.matmul(
                            ps[:, :],
                            wk_sb[:, dt, bass.ts(ht, 128)],
                            xk_sb[:, dt, ssl],
                            start=(dt == 0),
                            stop=(dt == NDT - 1),
                        )
                    nc.vector.tensor_scalar_add(
                        out=KT[:, ht, ssl],
                        in0=ps[:, :],
                        scalar1=bk_sb[:, ht : ht + 1],
                    )

            def emit_q_proj(qb):
                # QT[:, :, qb*512:+512] (all head pairs for one q-block)
                ssl = slice(qb * 512, (qb + 1) * 512)
                sb = slice((qb % 2) * 512, (qb % 2) * 512 + 512)
                xst = xq_sb[:, :, sb]
                for ht in range(NHT):
                    ps = ps_o.tile([128, 512], F32, tag="po")
                    for dt in range(NDT):
                        nc.tensor.matmul(
                            ps[:, :],
                            wq_sb[:, dt, bass.ts(ht, 128)],
                            xst[:, dt, :],
                            start=(dt == 0),
                            stop=(dt == NDT - 1),
                        )
                    nc.vector.tensor_scalar_add(
                        out=QT[:, ht, ssl],
                        in0=ps[:, :],
                        scalar1=bq_sb[:, ht : ht + 1],
                    )

            def emit_v_proj(b):
                # V natural layout, one s-block, all 8 heads
                for su in range(4):
                    ps = ps_o.tile([128, 512], F32, tag="po")
                    x0 = b * 512 + su * 128
                    for dt in range(NDT):
                        nc.tensor.matmul(
                            ps[:, :],
                            xv_sb[:, dt, x0 : x0 + 128],
                            wv_sb[:, dt, :],
                            start=(dt == 0),
                            stop=(dt == NDT - 1),
                        )
                    nc.vector.tensor_add(
                        out=Vt[:, b * 4 + su, :, 0:64],
                        in0=ps.rearrange("p (h d) -> p h d", d=64),
                        in1=bv_bc,
                    )

            def emit_sweep(qb, p):
                # attention for head pair p over q-block qb (512 queries)
                q0 = qb * 512
                hA, hB = 2 * p, 2 * p + 1
                ctxA = ps_ctx.tile([65, 512], F32, tag="ctx")
                ctxB = ps_ctx.tile([65, 512], F32, tag="ctx")
                for sk in range(NST):
                    st = ps_sc.tile([128, 1024], F32, tag="sc")
                    # concurrent pair: row groups 0:64 and 64:128
                    nc.tensor.matmul(
                        st[:, 0:512],
                        KT[0:64, p, bass.ts(sk, 128)],
                        QT[0:64, p, q0 : q0 + 512],
                        start=True,
                        stop=True,
                    )
                    nc.tensor.matmul(
                        st[:, 512:1024],
                        KT[64:128, p, bass.ts(sk, 128)],
                        QT[64:128, p, q0 : q0 + 512],
                        start=True,
                        stop=True,
                    )
                    pexp = pexp_p.tile([128, 1024], F16, tag="pexp")
                    nc.scalar.activation(
                        out=pexp, in_=st, func=AF.Exp, scale=0.125
                    )
                    nc.tensor.matmul(
                        ctxA,
                        Vt[:, sk, hA, :],
                        pexp[:, 0:512],
                        start=(sk == 0),
                        stop=(sk == NST - 1),
                    )
                    nc.tensor.matmul(
                        ctxB,
                        Vt[:, sk, hB, :],
                        pexp[:, 512:1024],
                        start=(sk == 0),
                        stop=(sk == NST - 1),
                    )
                # normalize both heads into ctxn[po:po+64, p, q0:q0+512]
                for ctx, po in ((ctxA, 0), (ctxB, 64)):
                    ctxc = small.tile([65, 512], F32, tag="ctxc")
                    nc.vector.tensor_copy(out=ctxc, in_=ctx[:, :])
                    rr = small.tile([128, 4], F32, tag="rr")
                    nc.sync.dma_start(out=rr, in_=ctxc[64:65, :])
                    rrv = small.tile([128, 4], F32, tag="rrv")
                    nc.vector.reciprocal(out=rrv, in_=rr)
                    rinvrow = small.tile([1, 512], F32, tag="rinvrow")
                    nc.sync.dma_start(out=rinvrow, in_=rrv)
                    rbc = small.tile([64, 512], F32, tag="rbc")
                    nc.gpsimd.partition_broadcast(rbc, rinvrow)
                    nc.vector.tensor_mul(
                        out=ctxn[po : po + 64, p, q0 : q0 + 512],
                        in0=ctxc[0:64, :],
                        in1=rbc,
                    )

            def emit_o_proj(qb):
                q0 = qb * 512
                for dot in range(8):
                    ps = ps_o.tile([128, 512], F32, tag="po")
                    for kt in range(NHT):
                        nc.tensor.matmul(
                            ps[:, :],
                            wo_sb[:, kt, bass.ts(dot, 128)],
                            ctxn[:, kt, q0 : q0 + 512],
                            start=(kt == 0),
                            stop=(kt == NHT - 1),
                        )
                    osb = outp.tile([128, 512], F32, tag="osb")
                    nc.vector.tensor_copy(out=osb, in_=ps)
                    nc.sync.dma_start(
                        out=ot_v[dot, :, q0 : q0 + 512], in_=osb
                    )

            # Projection emission order tracks the sweep schedule's needs:
            # sweeps run (qb0: p0..p3)(qb1: p0..p3)(qb2..)(qb3..); pair p
            # needs K_ht(p) full-S, V of its heads full-S, Q of the block.
            emit_k_proj(0)
            emit_v_proj(0)
            emit_v_proj(1)
            emit_q_proj(0)
            emit_v_proj(2)
            emit_q_proj(1)
            emit_v_proj(3)
            emit_k_proj(1)
            dma_xq(2)
            emit_q_proj(2)
            emit_k_proj(2)
            dma_xq(3)
            emit_q_proj(3)
            emit_k_proj(3)
            # round 1: qb 0-2 interleaved per pair (12 ACT-paced sweeps
            # absorb the K/V/Q front-load); round 2: qb3 + O-proj filler
            with tc.high_priority():
                for p in range(NHT):
                    emit_sweep(0, p)
                    emit_sweep(1, p)
                    emit_sweep(2, p)
            emit_o_proj(0)
            emit_o_proj(1)
            emit_o_proj(2)
            with tc.high_priority():
                for p in range(NHT):
                    emit_sweep(3, p)
            emit_o_proj(3)

    nc.compile()
    return nc


_NC_CACHE = None


def _get_nc():
    global _NC_CACHE
    if _NC_CACHE is None:
        _NC_CACHE = build_nc()
    return _NC_CACHE


def make_in_maps(q, k, v, Wq, bq, Wk, bk, Wv, bv, Wo):
    bf = np.float16
    in_maps = []
    for core in range(N_CORES):
        b, hg = core // 2, core % 2
        csl = slice(hg * DHT, (hg + 1) * DHT)
        in_maps.append(
            {
                "xq_t": np.ascontiguousarray(q[b].T).astype(bf),
                "xk_t": np.ascontiguousarray(k[b].T).astype(bf),
                "xv_t": np.ascontiguousarray(v[b].T).astype(bf),
                "wq": np.ascontiguousarray(Wq[:, csl]).astype(bf),
                "wk": np.ascontiguousarray(Wk[:, csl]).astype(bf),
                "wv": np.ascontiguousarray(Wv[:, csl]).astype(bf),
                "wo": np.ascontiguousarray(Wo[csl, :]).astype(bf),
                "bq": np.ascontiguousarray(bq[csl]).astype(np.float32),
                "bk": np.ascontiguousarray(bk[csl]).astype(np.float32),
                "bv": np.ascontiguousarray(bv[csl]).astype(np.float32),
            }
        )
    return in_maps


def kernel(q, k, v, Wq, bq, Wk, bk, Wv, bv, Wo, bo):
    q = np.asarray(q, np.float32)
    k = np.asarray(k, np.float32)
    v = np.asarray(v, np.float32)
    Wq = np.asarray(Wq, np.float32)
    Wk = np.asarray(Wk, np.float32)
    Wv = np.asarray(Wv, np.float32)
    Wo = np.asarray(Wo, np.float32)
    bq = np.asarray(bq, np.float32)
    bk = np.asarray(bk, np.float32)
    bv = np.asarray(bv, np.float32)
    bo = np.asarray(bo, np.float32)

    nc = _get_nc()
    in_maps = make_in_maps(q, k, v, Wq, bq, Wk, bk, Wv, bv, Wo)
    res = run_bass_kernel_spmd(nc, in_maps, list(range(N_CORES)))
    out = np.empty((B, S, D), np.float32)
    for b in range(B):
        o_t = res.results[2 * b]["o_t"] + res.results[2 * b + 1]["o_t"]
        out[b] = o_t.T + bo
    return out


# revision 7
# speedup vs baseline: 1.0085x; 1.0085x over previous
"""Multi-head attention (B=4, S=2048, D=1024, H=16) on 8 trn2 NeuronCores.

Sharding: (batch, head-group) -> 8 shards of (1 batch x 8 heads). Zero
cross-core communication: each core computes Q/K/V projections for its 8
heads, full attention over S=2048, and a partial output projection
(row-split Wo); the host sums the two head-group partials per batch.

v2 over the 454us baseline: attention processes HEAD PAIRS with the even
head's K/Q slice on SBUF partitions 0:64 and the odd head's on 64:128.
The two scores matmuls of a pair then carry tile_position (0,0) / (64,0)
(auto-derived from base_partition) and execute CONCURRENTLY on disjoint
PE row groups (measured 1.75x on a microbench), recovering the half-PE
waste of the K=dh=64 contraction. Each pair writes one [128, 1024] PSUM
tile = [scores_hA(512q) | scores_hB(512q)], so the exp ACT count stays
256 (ACT is the pacing engine at ~285us busy). Projections are emitted
ht-major (K) / pair-sliced (V) so each pair's sweep dependencies complete
just ahead of its sweep, with leftover projections + the output
projection filling PE slack under the ACT-paced attention phase.
"""

import numpy as np

import concourse.bass as bass
import concourse.tile as tile
from concourse import bacc, mybir
from concourse.bass_utils import run_bass_kernel_spmd

F32 = mybir.dt.float32
F16 = mybir.dt.float16
AF = mybir.ActivationFunctionType

B, S, D = 4, 2048, 1024
HPC = 8          # heads per core
DHT = 512        # head dims per core (8 * 64)
NDT = D // 128   # 8 d-tiles (contraction tiles for projections)
NHT = DHT // 128  # 4 dh-tiles (= head pairs)
NST = S // 128   # 16 s-tiles
NQB = S // 512   # 4 q-blocks
N_CORES = 8


def build_nc():
    nc = bacc.Bacc(None, target_bir_lowering=False)

    xq = nc.declare_dram_parameter("xq_t", [D, S], F16, isOutput=False)
    xk = nc.declare_dram_parameter("xk_t", [D, S], F16, isOutput=False)
    xv = nc.declare_dram_parameter("xv_t", [D, S], F16, isOutput=False)
    wq = nc.declare_dram_parameter("wq", [D, DHT], F16, isOutput=False)
    wk = nc.declare_dram_parameter("wk", [D, DHT], F16, isOutput=False)
    wv = nc.declare_dram_parameter("wv", [D, DHT], F16, isOutput=False)
    wo = nc.declare_dram_parameter("wo", [DHT, D], F16, isOutput=False)
    bq = nc.declare_dram_parameter("bq", [DHT], F32, isOutput=False)
    bk = nc.declare_dram_parameter("bk", [DHT], F32, isOutput=False)
    bv = nc.declare_dram_parameter("bv", [DHT], F32, isOutput=False)
    ot = nc.declare_dram_parameter("o_t", [D, S], F32, isOutput=True)

    xq_v = xq.rearrange("(t p) s -> p t s", p=128)
    xk_v = xk.rearrange("(t p) s -> p t s", p=128)
    xv_v = xv.rearrange("(t p) s -> p t s", p=128)
    wq_v = wq.rearrange("(t p) n -> p t n", p=128)
    wk_v = wk.rearrange("(t p) n -> p t n", p=128)
    wv_v = wv.rearrange("(t p) n -> p t n", p=128)
    wo_v = wo.rearrange("(t p) n -> p t n", p=128)
    ot_v = ot.rearrange("(t p) s -> t p s", p=128)

    with tile.TileContext(nc) as tc:
        with (
            tc.tile_pool(name="persist", bufs=1) as persist,
            tc.tile_pool(name="pexp_p", bufs=4) as pexp_p,
            tc.tile_pool(name="outp", bufs=3) as outp,
            tc.tile_pool(name="small", bufs=2) as small,
            tc.tile_pool(name="ps_sc", bufs=2, space="PSUM") as ps_sc,
            tc.tile_pool(name="ps_ctx", bufs=2, space="PSUM") as ps_ctx,
            tc.tile_pool(name="ps_o", bufs=2, space="PSUM") as ps_o,
        ):
            KT = persist.tile([128, NHT, S], F16)        # K^T  [dh, s]
            QT = persist.tile([128, NHT, S], F16)        # Q^T  [dh, s]
            Vt = persist.tile([128, NST, HPC, 65], F16)  # V natural + ones
            ctxn = persist.tile([128, NHT, S], F16)      # normalized ctx^T
            wq_sb = persist.tile([128, NDT, DHT], F16)
            wk_sb = persist.tile([128, NDT, DHT], F16)
            wv_sb = persist.tile([128, NDT, DHT], F16)
            wo_sb = persist.tile([128, NHT, D], F16)
            xk_sb = persist.tile([128, NDT, S], F16)
            xv_sb = persist.tile([128, NDT, S], F16)
            xq_sb = persist.tile([128, NDT, 1024], F16)
            bq_sb = persist.tile([128, NHT], F32)
            bk_sb = persist.tile([128, NHT], F32)
            bv_bc = persist.tile([128, HPC, 64], F32)

            # DMA issue spread over 4 engine queues so the prefix
            # transfers parallelize: scalar=wk/wq/wo, vector=wv+biases,
            # sync=xk+xq (block-major), gpsimd=xv (block-major)
            nc.sync.dma_start(out=bq_sb, in_=bq.rearrange("(t p) -> p t", p=128))
            nc.sync.dma_start(out=bk_sb, in_=bk.rearrange("(t p) -> p t", p=128))
            nc.sync.dma_start(
                out=bv_bc,
                in_=bv.rearrange("(h d) -> h d", d=64).partition_broadcast(128),
            )
            nc.vector.memset(Vt[:, :, :, 64:65], 1.0)
            for dt in range(NDT):
                nc.scalar.dma_start(out=wk_sb[:, dt, :], in_=wk_v[:, dt, :])
            for dt in range(NDT):
                nc.gpsimd.dma_start(out=wv_sb[:, dt, :], in_=wv_v[:, dt, :])
            for dt in range(NDT):
                nc.scalar.dma_start(out=wq_sb[:, dt, :], in_=wq_v[:, dt, :])

            def dma_xk(b):
                ssl = slice(b * 512, (b + 1) * 512)
                for dt in range(NDT):
                    nc.sync.dma_start(out=xk_sb[:, dt, ssl], in_=xk_v[:, dt, ssl])

            def dma_xv(b):
                ssl = slice(b * 512, (b + 1) * 512)
                for dt in range(NDT):
                    nc.gpsimd.dma_start(out=xv_sb[:, dt, ssl], in_=xv_v[:, dt, ssl])

            def dma_xq(qb):
                ssl = slice(qb * 512, (qb + 1) * 512)
                sb = slice((qb % 2) * 512, (qb % 2) * 512 + 512)
                for dt in range(NDT):
                    nc.sync.dma_start(out=xq_sb[:, dt, sb], in_=xq_v[:, dt, ssl])

            dma_xk(0)
            dma_xk(1)
            dma_xq(0)
            dma_xk(2)
            dma_xk(3)
            dma_xq(1)
            for b in range(4):
                dma_xv(b)
            for kt in range(NHT):
                nc.gpsimd.dma_start(out=wo_sb[:, kt, :], in_=wo_v[:, kt, :])

            def emit_k_proj(ht):
                # KT[:, ht, :] for head pair ht (dh rows 0:64 / 64:128)
                for b in range(4):
                    ssl = slice(b * 512, (b + 1) * 512)
                    ps = ps_o.tile([128, 512], F32, tag="po")
                    for dt in range(NDT):
                        nc.tensor.matmul(
                            ps[:, :],
                            wk_sb[:, dt, bass.ts(ht, 128)],
                            xk_sb[:, dt, ssl],
                            start=(dt == 0),
                            stop=(dt == NDT - 1),
                        )
                    nc.vector.tensor_scalar_add(
                        out=KT[:, ht, ssl],
                        in0=ps[:, :],
                        scalar1=bk_sb[:, ht : ht + 1],
                    )

            def emit_q_proj(qb):
                # QT[:, :, qb*512:+512] (all head pairs for one q-block)
                ssl = slice(qb * 512, (qb + 1) * 512)
                sb = slice((qb % 2) * 512, (qb % 2) * 512 + 512)
                xst = xq_sb[:, :, sb]
                for ht in range(NHT):
                    ps = ps_o.tile([128, 512], F32, tag="po")
                    for dt in range(NDT):
                        nc.tensor.matmul(
                            ps[:, :],
                            wq_sb[:, dt, bass.ts(ht, 128)],
                            xst[:, dt, :],
                            start=(dt == 0),
                            stop=(dt == NDT - 1),
                        )
                    nc.vector.tensor_scalar_add(
                        out=QT[:, ht, ssl],
                        in0=ps[:, :],
                        scalar1=bq_sb[:, ht : ht + 1],
                    )

            def emit_v_proj(b):
                # V natural layout, one s-block, all 8 heads
                for su in range(4):
                    ps = ps_o.tile([128, 512], F32, tag="po")
                    x0 = b * 512 + su * 128
                    for dt in range(NDT):
                        nc.tensor.matmul(
                            ps[:, :],
                            xv_sb[:, dt, x0 : x0 + 128],
                            wv_sb[:, dt, :],
                            start=(dt == 0),
                            stop=(dt == NDT - 1),
                        )
                    nc.vector.tensor_add(
                        out=Vt[:, b * 4 + su, :, 0:64],
                        in0=ps.rearrange("p (h d) -> p h d", d=64),
                        in1=bv_bc,
                    )

            def emit_sweep(qb, p):
                # attention for head pair p over q-block qb (512 queries)
                q0 = qb * 512
                hA, hB = 2 * p, 2 * p + 1
                ctxA = ps_ctx.tile([65, 512], F32, tag="ctx")
                ctxB = ps_ctx.tile([65, 512], F32, tag="ctx")
                for sk in range(NST):
                    st = ps_sc.tile([128, 1024], F32, tag="sc")
                    # concurrent pair: row groups 0:64 and 64:128
                    nc.tensor.matmul(
                        st[:, 0:512],
                        KT[0:64, p, bass.ts(sk, 128)],
                        QT[0:64, p, q0 : q0 + 512],
                        start=True,
                        stop=True,
                    )
                    nc.tensor.matmul(
                        st[:, 512:1024],
                        KT[64:128, p, bass.ts(sk, 128)],
                        QT[64:128, p, q0 : q0 + 512],
                        start=True,
                        stop=True,
                    )
                    pexp = pexp_p.tile([128, 1024], F16, tag="pexp")
                    nc.scalar.activation(
                        out=pexp, in_=st, func=AF.Exp, scale=0.125
                    )
                    nc.tensor.matmul(
                        ctxA,
                        Vt[:, sk, hA, :],
                        pexp[:, 0:512],
                        start=(sk == 0),
                        stop=(sk == NST - 1),
                    )
                    nc.tensor.matmul(
                        ctxB,
                        Vt[:, sk, hB, :],
                        pexp[:, 512:1024],
                        start=(sk == 0),
                        stop=(sk == NST - 1),
                    )
                # normalize both heads into ctxn[po:po+64, p, q0:q0+512]
                for ctx, po in ((ctxA, 0), (ctxB, 64)):
                    ctxc = small.tile([65, 512], F32, tag="ctxc")
                    nc.vector.tensor_copy(out=ctxc, in_=ctx[:, :])
                    rr = small.tile([128, 4], F32, tag="rr")
                    nc.sync.dma_start(out=rr, in_=ctxc[64:65, :])
                    rrv = small.tile([128, 4], F32, tag="rrv")
                    nc.vector.reciprocal(out=rrv, in_=rr)
                    rinvrow = small.tile([1, 512], F32, tag="rinvrow")
                    nc.sync.dma_start(out=rinvrow, in_=rrv)
                    rbc = small.tile([64, 512], F32, tag="rbc")
                    nc.gpsimd.partition_broadcast(rbc, rinvrow)
                    nc.vector.tensor_mul(
                        out=ctxn[po : po + 64, p, q0 : q0 + 512],
                        in0=ctxc[0:64, :],
                        in1=rbc,
                    )

            def emit_o_proj(qb):
                q0 = qb * 512
                for dot in range(8):
                    ps = ps_o.tile([128, 512], F32, tag="po")
                    for kt in range(NHT):
                        nc.tensor.matmul(
                            ps[:, :],
                            wo_sb[:, kt, bass.ts(dot, 128)],
                            ctxn[:, kt, q0 : q0 + 512],
                            start=(kt == 0),
                            stop=(kt == NHT - 1),
                        )
                    osb = outp.tile([128, 512], F32, tag="osb")
                    nc.vector.tensor_copy(out=osb, in_=ps)
                    nc.sync.dma_start(
                        out=ot_v[dot, :, q0 : q0 + 512], in_=osb
                    )

            # Projection emission order tracks the sweep schedule's needs:
            # sweeps run (qb0: p0..p3)(qb1: p0..p3)(qb2..)(qb3..); pair p
            # needs K_ht(p) full-S, V of its heads full-S, Q of the block.
            emit_k_proj(0)
            emit_v_proj(0)
            emit_v_proj(1)
            emit_q_proj(0)
            emit_v_proj(2)
            emit_q_proj(1)
            emit_v_proj(3)
            emit_k_proj(1)
            dma_xq(2)
            emit_q_proj(2)
            emit_k_proj(2)
            dma_xq(3)
            emit_q_proj(3)
            emit_k_proj(3)
            # round 1: qb 0-2 interleaved per pair (12 ACT-paced sweeps
            # absorb the K/V/Q front-load); round 2: qb3 + O-proj filler
            with tc.high_priority():
                for p in range(NHT):
                    emit_sweep(0, p)
                    emit_sweep(1, p)
                    emit_sweep(2, p)
            emit_o_proj(0)
            emit_o_proj(1)
            emit_o_proj(2)
            with tc.high_priority():
                for p in range(NHT):
                    emit_sweep(3, p)
            emit_o_proj(3)

    nc.compile()
    return nc


_NC_CACHE = None


def _get_nc():
    global _NC_CACHE
    if _NC_CACHE is None:
        _NC_CACHE = build_nc()
    return _NC_CACHE


def make_in_maps(q, k, v, Wq, bq, Wk, bk, Wv, bv, Wo):
    bf = np.float16
    in_maps = []
    for core in range(N_CORES):
        b, hg = core // 2, core % 2
        csl = slice(hg * DHT, (hg + 1) * DHT)
        in_maps.append(
            {
                "xq_t": np.ascontiguousarray(q[b].T).astype(bf),
                "xk_t": np.ascontiguousarray(k[b].T).astype(bf),
                "xv_t": np.ascontiguousarray(v[b].T).astype(bf),
                "wq": np.ascontiguousarray(Wq[:, csl]).astype(bf),
                "wk": np.ascontiguousarray(Wk[:, csl]).astype(bf),
                "wv": np.ascontiguousarray(Wv[:, csl]).astype(bf),
                "wo": np.ascontiguousarray(Wo[csl, :]).astype(bf),
                "bq": np.ascontiguousarray(bq[csl]).astype(np.float32),
                "bk": np.ascontiguousarray(bk[csl]).astype(np.float32),
                "bv": np.ascontiguousarray(bv[csl]).astype(np.float32),
            }
        )
    return in_maps


def kernel(q, k, v, Wq, bq, Wk, bk, Wv, bv, Wo, bo):
    q = np.asarray(q, np.float32)
    k = np.asarray(k, np.float32)
    v = np.asarray(v, np.float32)
    Wq = np.asarray(Wq, np.float32)
    Wk = np.asarray(Wk, np.float32)
    Wv = np.asarray(Wv, np.float32)
    Wo = np.asarray(Wo, np.float32)
    bq = np.asarray(bq, np.float32)
    bk = np.asarray(bk, np.float32)
    bv = np.asarray(bv, np.float32)
    bo = np.asarray(bo, np.float32)

    nc = _get_nc()
    in_maps = make_in_maps(q, k, v, Wq, bq, Wk, bk, Wv, bv, Wo)
    res = run_bass_kernel_spmd(nc, in_maps, list(range(N_CORES)))
    out = np.empty((B, S, D), np.float32)
    for b in range(B):
        o_t = res.results[2 * b]["o_t"] + res.results[2 * b + 1]["o_t"]
        out[b] = o_t.T + bo
    return out


# revision 8
# speedup vs baseline: 1.0154x; 1.0069x over previous
"""Multi-head attention (B=4, S=2048, D=1024, H=16) on 8 trn2 NeuronCores.

Sharding: (batch, head-group) -> 8 shards of (1 batch x 8 heads). Zero
cross-core communication: each core computes Q/K/V projections for its 8
heads, full attention over S=2048, and a partial output projection
(row-split Wo); the host sums the two head-group partials per batch.

v2 over the 454us baseline: attention processes HEAD PAIRS with the even
head's K/Q slice on SBUF partitions 0:64 and the odd head's on 64:128.
The two scores matmuls of a pair then carry tile_position (0,0) / (64,0)
(auto-derived from base_partition) and execute CONCURRENTLY on disjoint
PE row groups (measured 1.75x on a microbench), recovering the half-PE
waste of the K=dh=64 contraction. Each pair writes one [128, 1024] PSUM
tile = [scores_hA(512q) | scores_hB(512q)], so the exp ACT count stays
256 (ACT is the pacing engine at ~285us busy). Projections are emitted
ht-major (K) / pair-sliced (V) so each pair's sweep dependencies complete
just ahead of its sweep, with leftover projections + the output
projection filling PE slack under the ACT-paced attention phase.
"""

import numpy as np

import concourse.bass as bass
import concourse.tile as tile
from concourse import bacc, mybir
from concourse.bass_utils import run_bass_kernel_spmd

F32 = mybir.dt.float32
F16 = mybir.dt.float16
AF = mybir.ActivationFunctionType

B, S, D = 4, 2048, 1024
HPC = 8          # heads per core
DHT = 512        # head dims per core (8 * 64)
NDT = D // 128   # 8 d-tiles (contraction tiles for projections)
NHT = DHT // 128  # 4 dh-tiles (= head pairs)
NST = S // 128   # 16 s-tiles
NQB = S // 512   # 4 q-blocks
N_CORES = 8


def build_nc():
    nc = bacc.Bacc(None, target_bir_lowering=False)

    xq = nc.declare_dram_parameter("xq_t", [D, S], F16, isOutput=False)
    xk = nc.declare_dram_parameter("xk_t", [D, S], F16, isOutput=False)
    xv = nc.declare_dram_parameter("xv_t", [D, S], F16, isOutput=False)
    wq = nc.declare_dram_parameter("wq", [D, DHT], F16, isOutput=False)
    wk = nc.declare_dram_parameter("wk", [D, DHT], F16, isOutput=False)
    wv = nc.declare_dram_parameter("wv", [D, DHT], F16, isOutput=False)
    wo = nc.declare_dram_parameter("wo", [DHT, D], F16, isOutput=False)
    bq = nc.declare_dram_parameter("bq", [DHT], F32, isOutput=False)
    bk = nc.declare_dram_parameter("bk", [DHT], F32, isOutput=False)
    bv = nc.declare_dram_parameter("bv", [DHT], F32, isOutput=False)
    ot = nc.declare_dram_parameter("o_t", [D, S], F32, isOutput=True)

    xq_v = xq.rearrange("(t p) s -> p t s", p=128)
    xk_v = xk.rearrange("(t p) s -> p t s", p=128)
    xv_v = xv.rearrange("(t p) s -> p t s", p=128)
    wq_v = wq.rearrange("(t p) n -> p t n", p=128)
    wk_v = wk.rearrange("(t p) n -> p t n", p=128)
    wv_v = wv.rearrange("(t p) n -> p t n", p=128)
    wo_v = wo.rearrange("(t p) n -> p t n", p=128)
    ot_v = ot.rearrange("(t p) s -> t p s", p=128)

    with tile.TileContext(nc) as tc:
        with (
            tc.tile_pool(name="persist", bufs=1) as persist,
            tc.tile_pool(name="pexp_p", bufs=4) as pexp_p,
            tc.tile_pool(name="outp", bufs=3) as outp,
            tc.tile_pool(name="small", bufs=2) as small,
            tc.tile_pool(name="ps_sc", bufs=2, space="PSUM") as ps_sc,
            tc.tile_pool(name="ps_ctx", bufs=2, space="PSUM") as ps_ctx,
            tc.tile_pool(name="ps_o", bufs=2, space="PSUM") as ps_o,
        ):
            KT = persist.tile([128, NHT, S], F16)        # K^T  [dh, s]
            QT = persist.tile([128, NHT, S], F16)        # Q^T  [dh, s]
            Vt = persist.tile([128, NST, HPC, 65], F16)  # V natural + ones
            ctxn = persist.tile([128, NHT, S], F16)      # normalized ctx^T
            wq_sb = persist.tile([128, NDT, DHT], F16)
            wk_sb = persist.tile([128, NDT, DHT], F16)
            wv_sb = persist.tile([128, NDT, DHT], F16)
            wo_sb = persist.tile([128, NHT, D], F16)
            xk_sb = persist.tile([128, NDT, S], F16)
            xv_sb = persist.tile([128, NDT, S], F16)
            xq_sb = persist.tile([128, NDT, 1024], F16)
            bq_sb = persist.tile([128, NHT], F32)
            bk_sb = persist.tile([128, NHT], F32)
            bv_bc = persist.tile([128, HPC, 64], F32)

            # DMA issue spread over 4 engine queues so the prefix
            # transfers parallelize: scalar=wk/wq/wo, vector=wv+biases,
            # sync=xk+xq (block-major), gpsimd=xv (block-major)
            nc.sync.dma_start(out=bq_sb, in_=bq.rearrange("(t p) -> p t", p=128))
            nc.sync.dma_start(out=bk_sb, in_=bk.rearrange("(t p) -> p t", p=128))
            nc.sync.dma_start(
                out=bv_bc,
                in_=bv.rearrange("(h d) -> h d", d=64).partition_broadcast(128),
            )
            nc.vector.memset(Vt[:, :, :, 64:65], 1.0)
            for dt in range(NDT):
                nc.scalar.dma_start(out=wk_sb[:, dt, :], in_=wk_v[:, dt, :])
            for dt in range(NDT):
                nc.gpsimd.dma_start(out=wv_sb[:, dt, :], in_=wv_v[:, dt, :])
            for dt in range(NDT):
                nc.scalar.dma_start(out=wq_sb[:, dt, :], in_=wq_v[:, dt, :])

            def dma_xk(b):
                ssl = slice(b * 512, (b + 1) * 512)
                for dt in range(NDT):
                    nc.sync.dma_start(out=xk_sb[:, dt, ssl], in_=xk_v[:, dt, ssl])

            def dma_xv(b):
                ssl = slice(b * 512, (b + 1) * 512)
                for dt in range(NDT):
                    nc.gpsimd.dma_start(out=xv_sb[:, dt, ssl], in_=xv_v[:, dt, ssl])

            def dma_xq(qb, eng=None):
                eng = eng or nc.sync
                ssl = slice(qb * 512, (qb + 1) * 512)
                sb = slice((qb % 2) * 512, (qb % 2) * 512 + 512)
                for dt in range(NDT):
                    eng.dma_start(out=xq_sb[:, dt, sb], in_=xq_v[:, dt, ssl])

            dma_xk(0)
            dma_xk(1)
            dma_xq(0)
            dma_xk(2)
            dma_xk(3)
            dma_xq(1)
            for b in range(4):
                dma_xv(b)
            for kt in range(NHT):
                nc.gpsimd.dma_start(out=wo_sb[:, kt, :], in_=wo_v[:, kt, :])

            def emit_k_proj(ht):
                # KT[:, ht, :] for head pair ht (dh rows 0:64 / 64:128)
                for b in range(4):
                    ssl = slice(b * 512, (b + 1) * 512)
                    ps = ps_o.tile([128, 512], F32, tag="po")
                    for dt in range(NDT):
                        nc.tensor.matmul(
                            ps[:, :],
                            wk_sb[:, dt, bass.ts(ht, 128)],
                            xk_sb[:, dt, ssl],
                            start=(dt == 0),
                            stop=(dt == NDT - 1),
                        )
                    nc.vector.tensor_scalar_add(
                        out=KT[:, ht, ssl],
                        in0=ps[:, :],
                        scalar1=bk_sb[:, ht : ht + 1],
                    )

            def emit_q_proj(qb):
                # QT[:, :, qb*512:+512] (all head pairs for one q-block)
                ssl = slice(qb * 512, (qb + 1) * 512)
                sb = slice((qb % 2) * 512, (qb % 2) * 512 + 512)
                xst = xq_sb[:, :, sb]
                for ht in range(NHT):
                    ps = ps_o.tile([128, 512], F32, tag="po")
                    for dt in range(NDT):
                        nc.tensor.matmul(
                            ps[:, :],
                            wq_sb[:, dt, bass.ts(ht, 128)],
                            xst[:, dt, :],
                            start=(dt == 0),
                            stop=(dt == NDT - 1),
                        )
                    nc.vector.tensor_scalar_add(
                        out=QT[:, ht, ssl],
                        in0=ps[:, :],
                        scalar1=bq_sb[:, ht : ht + 1],
                    )

            def emit_v_proj(b):
                # V natural layout, one s-block, all 8 heads
                for su in range(4):
                    ps = ps_o.tile([128, 512], F32, tag="po")
                    x0 = b * 512 + su * 128
                    for dt in range(NDT):
                        nc.tensor.matmul(
                            ps[:, :],
                            xv_sb[:, dt, x0 : x0 + 128],
                            wv_sb[:, dt, :],
                            start=(dt == 0),
                            stop=(dt == NDT - 1),
                        )
                    nc.vector.tensor_add(
                        out=Vt[:, b * 4 + su, :, 0:64],
                        in0=ps.rearrange("p (h d) -> p h d", d=64),
                        in1=bv_bc,
                    )

            def emit_sweep(qb, p):
                # attention for head pair p over q-block qb (512 queries)
                q0 = qb * 512
                hA, hB = 2 * p, 2 * p + 1
                ctxA = ps_ctx.tile([65, 512], F32, tag="ctx")
                ctxB = ps_ctx.tile([65, 512], F32, tag="ctx")
                for sk in range(NST):
                    st = ps_sc.tile([128, 1024], F32, tag="sc")
                    # concurrent pair: row groups 0:64 and 64:128
                    nc.tensor.matmul(
                        st[:, 0:512],
                        KT[0:64, p, bass.ts(sk, 128)],
                        QT[0:64, p, q0 : q0 + 512],
                        start=True,
                        stop=True,
                    )
                    nc.tensor.matmul(
                        st[:, 512:1024],
                        KT[64:128, p, bass.ts(sk, 128)],
                        QT[64:128, p, q0 : q0 + 512],
                        start=True,
                        stop=True,
                    )
                    pexp = pexp_p.tile([128, 1024], F16, tag="pexp")
                    nc.scalar.activation(
                        out=pexp, in_=st, func=AF.Exp, scale=0.125
                    )
                    nc.tensor.matmul(
                        ctxA,
                        Vt[:, sk, hA, :],
                        pexp[:, 0:512],
                        start=(sk == 0),
                        stop=(sk == NST - 1),
                    )
                    nc.tensor.matmul(
                        ctxB,
                        Vt[:, sk, hB, :],
                        pexp[:, 512:1024],
                        start=(sk == 0),
                        stop=(sk == NST - 1),
                    )
                # normalize both heads into ctxn[po:po+64, p, q0:q0+512]
                for ctx, po in ((ctxA, 0), (ctxB, 64)):
                    ctxc = small.tile([65, 512], F32, tag="ctxc")
                    nc.vector.tensor_copy(out=ctxc, in_=ctx[:, :])
                    rr = small.tile([128, 4], F32, tag="rr")
                    nc.sync.dma_start(out=rr, in_=ctxc[64:65, :])
                    rrv = small.tile([128, 4], F32, tag="rrv")
                    nc.vector.reciprocal(out=rrv, in_=rr)
                    rinvrow = small.tile([1, 512], F32, tag="rinvrow")
                    nc.sync.dma_start(out=rinvrow, in_=rrv)
                    rbc = small.tile([64, 512], F32, tag="rbc")
                    nc.gpsimd.partition_broadcast(rbc, rinvrow)
                    nc.vector.tensor_mul(
                        out=ctxn[po : po + 64, p, q0 : q0 + 512],
                        in0=ctxc[0:64, :],
                        in1=rbc,
                    )

            def emit_o_proj(qb):
                q0 = qb * 512
                for dot in range(8):
                    ps = ps_o.tile([128, 512], F32, tag="po")
                    for kt in range(NHT):
                        nc.tensor.matmul(
                            ps[:, :],
                            wo_sb[:, kt, bass.ts(dot, 128)],
                            ctxn[:, kt, q0 : q0 + 512],
                            start=(kt == 0),
                            stop=(kt == NHT - 1),
                        )
                    osb = outp.tile([128, 512], F32, tag="osb")
                    nc.vector.tensor_copy(out=osb, in_=ps)
                    nc.sync.dma_start(
                        out=ot_v[dot, :, q0 : q0 + 512], in_=osb
                    )

            # Projection emission order tracks the sweep schedule's needs:
            # sweeps run (qb0: p0..p3)(qb1: p0..p3)(qb2..)(qb3..); pair p
            # needs K_ht(p) full-S, V of its heads full-S, Q of the block.
            emit_k_proj(0)
            emit_v_proj(0)
            emit_v_proj(1)
            emit_q_proj(0)
            emit_v_proj(2)
            emit_q_proj(1)
            emit_v_proj(3)
            emit_k_proj(1)
            emit_k_proj(2)
            emit_k_proj(3)
            dma_xq(2, nc.gpsimd)
            emit_q_proj(2)
            dma_xq(3, nc.gpsimd)
            emit_q_proj(3)
            # round 1: qb 0-1 interleaved per pair; round 2: qb 2-3
            with tc.high_priority():
                for p in range(NHT):
                    emit_sweep(0, p)
                    emit_sweep(1, p)
            emit_o_proj(0)
            emit_o_proj(1)
            with tc.high_priority():
                for p in range(NHT):
                    emit_sweep(2, p)
                    emit_sweep(3, p)
            emit_o_proj(2)
            emit_o_proj(3)

    nc.compile()
    return nc


_NC_CACHE = None


def _get_nc():
    global _NC_CACHE
    if _NC_CACHE is None:
        _NC_CACHE = build_nc()
    return _NC_CACHE


def make_in_maps(q, k, v, Wq, bq, Wk, bk, Wv, bv, Wo):
    bf = np.float16
    in_maps = []
    for core in range(N_CORES):
        b, hg = core // 2, core % 2
        csl = slice(hg * DHT, (hg + 1) * DHT)
        in_maps.append(
            {
                "xq_t": np.ascontiguousarray(q[b].T).astype(bf),
                "xk_t": np.ascontiguousarray(k[b].T).astype(bf),
                "xv_t": np.ascontiguousarray(v[b].T).astype(bf),
                "wq": np.ascontiguousarray(Wq[:, csl]).astype(bf),
                "wk": np.ascontiguousarray(Wk[:, csl]).astype(bf),
                "wv": np.ascontiguousarray(Wv[:, csl]).astype(bf),
                "wo": np.ascontiguousarray(Wo[csl, :]).astype(bf),
                "bq": np.ascontiguousarray(bq[csl]).astype(np.float32),
                "bk": np.ascontiguousarray(bk[csl]).astype(np.float32),
                "bv": np.ascontiguousarray(bv[csl]).astype(np.float32),
            }
        )
    return in_maps


def kernel(q, k, v, Wq, bq, Wk, bk, Wv, bv, Wo, bo):
    q = np.asarray(q, np.float32)
    k = np.asarray(k, np.float32)
    v = np.asarray(v, np.float32)
    Wq = np.asarray(Wq, np.float32)
    Wk = np.asarray(Wk, np.float32)
    Wv = np.asarray(Wv, np.float32)
    Wo = np.asarray(Wo, np.float32)
    bq = np.asarray(bq, np.float32)
    bk = np.asarray(bk, np.float32)
    bv = np.asarray(bv, np.float32)
    bo = np.asarray(bo, np.float32)

    nc = _get_nc()
    in_maps = make_in_maps(q, k, v, Wq, bq, Wk, bk, Wv, bv, Wo)
    res = run_bass_kernel_spmd(nc, in_maps, list(range(N_CORES)))
    out = np.empty((B, S, D), np.float32)
    for b in range(B):
        o_t = res.results[2 * b]["o_t"] + res.results[2 * b + 1]["o_t"]
        out[b] = o_t.T + bo
    return out
